# revision 10
# baseline (speedup 1.0000x reference)
"""Trainium2 Bass kernel for AttentiveTransformer:
   out = sparsemax(GBN(feat @ W.T) * priors)

Data-parallel over 8 NeuronCores: batch 131072 rows -> 8 shards of 16384.
Per core: 32 superchunks x 512 rows; each superchunk = 4 GBN chunks (VBS=128).

Pipeline (per superchunk):
  - one batched DMA each for feat/priors/out per superchunk
  - feat -> PE transpose -> featT [128k, 512r] (SBUF)
  - per d-slice s: PE matmul x_s = WT_s.T @ featT -> PSUM [128d, 512r]
  - DVE bn_stats per (slice, chunk) -> even/odd count/mean/M2; gpsimd merges
    and computes scale = gamma*rsqrt(var+eps), shift = beta - mean*scale
  - ACT per (s, chunk): xn = x*scale + shift (per-partition scalars)
  - per chunk: PE transpose back -> zT [128r, 512d] PSUM; DVE z = zT * priors
  - sparsemax via hardware top-8: 4x max8 on 128-col segments -> 32 cands,
    max8 -> top8, suppress, max8 -> next8 => sorted top-16 (support <= 15),
    closed-form tau from cumsum (tensor_tensor_scan) + prefix condition,
    ACT final: out = relu(z - tau)
"""
import sys

sys.path.insert(0, "/opt/trn_rl_repo")

import numpy as np
from contextlib import ExitStack

import concourse.bass as bass
import concourse.bacc as bacc
import concourse.tile as tile
from concourse.tile import add_dep_helper
from concourse import mybir
from concourse.bass_utils import run_bass_kernel_spmd

f32 = mybir.dt.float32
f32r = mybir.dt.float32r
AF = mybir.ActivationFunctionType
OP = mybir.AluOpType

N_CORES = 8
B, IN, D = 131072, 128, 512
ROWS = B // N_CORES          # 16384 rows per core
SC_ROWS = 512                # superchunk rows (4 GBN chunks)
N_SC = ROWS // SC_ROWS       # 32
VBS = 128
EPS = 1e-5
NEG = -1.0e9


def build_nc():
    nc = bacc.Bacc(None, target_bir_lowering=False)

    priors = nc.dram_tensor("priors", [ROWS, D], f32, kind="ExternalInput")
    feat = nc.dram_tensor("processed_feat", [ROWS, IN], f32, kind="ExternalInput")
    Wd = nc.dram_tensor("W", [D, IN], f32, kind="ExternalInput")
    gam = nc.dram_tensor("gamma", [D], f32, kind="ExternalInput")
    bet = nc.dram_tensor("beta", [D], f32, kind="ExternalInput")
    out = nc.dram_tensor("out", [ROWS, D], f32, kind="ExternalOutput")

    with tile.TileContext(nc) as tc, ExitStack() as ctx:
        singles = ctx.enter_context(tc.tile_pool(name="singles", bufs=1))
        ft_pool = ctx.enter_context(tc.tile_pool(name="ft", bufs=3))
        xn_pool = ctx.enter_context(tc.tile_pool(name="xn", bufs=12))
        xs_pool = ctx.enter_context(tc.tile_pool(name="xs", bufs=10))
        p_pool = ctx.enter_context(tc.tile_pool(name="p", bufs=4))
        z_pool = ctx.enter_context(tc.tile_pool(name="z", bufs=10))
        o_pool = ctx.enter_context(tc.tile_pool(name="o", bufs=4))
        st_pool = ctx.enter_context(tc.tile_pool(name="st", bufs=4))
        sm_pool = ctx.enter_context(tc.tile_pool(name="sm", bufs=24))
        ps_ft = ctx.enter_context(tc.tile_pool(name="psft", bufs=2, space="PSUM"))
        ps_x = ctx.enter_context(tc.tile_pool(name="psx", bufs=3, space="PSUM"))
        ps_zt = ctx.enter_context(tc.tile_pool(name="pszt", bufs=3, space="PSUM"))

        # ---------- one-time constants ----------
        ident = singles.tile([128, 128], f32)
        nc.gpsimd.iota(ident, [[1, 128]], base=0, channel_multiplier=-1,
                       allow_small_or_imprecise_dtypes=True)
        nc.vector.tensor_scalar(ident, ident, 0.0, None, OP.is_equal)

        # WT [128k, 512d] resident (f32r so the fp32r gemm gets pre-rounded
        # operands; the ACT copy below performs the rounding)
        WT = singles.tile([128, D], f32r)
        wtp = ps_ft.tile([128, D], f32, tag="ftp")
        for s in range(4):
            wtile = ft_pool.tile([128, 128], f32, tag="wtile")
            nc.sync.dma_start(out=wtile, in_=Wd[s * 128:(s + 1) * 128, :])
            nc.tensor.transpose(wtp[:, s * 128:(s + 1) * 128], wtile, ident)
        nc.scalar.copy(WT, wtp)

        # gamma/beta broadcast [128, 4slice, 4chunk]
        gamma44 = singles.tile([128, 4, 4], f32)
        beta44 = singles.tile([128, 4, 4], f32)
        gamma4 = singles.tile([128, 4], f32)
        beta4 = singles.tile([128, 4], f32)
        gr = gam.rearrange("(s p) -> s p", p=128)
        br = bet.rearrange("(s p) -> s p", p=128)
        for s4 in range(4):
            nc.sync.dma_start(out=gamma4[:, s4:s4 + 1],
                              in_=gr[s4].rearrange("(p o) -> p o", o=1))
            nc.sync.dma_start(out=beta4[:, s4:s4 + 1],
                              in_=br[s4].rearrange("(p o) -> p o", o=1))
        for c4 in range(4):
            nc.vector.tensor_copy(gamma44[:, :, c4], gamma4)
            nc.vector.tensor_copy(beta44[:, :, c4], beta4)

        eps_t = singles.tile([128, 1], f32)
        nc.vector.memset(eps_t, EPS)
        c32 = singles.tile([128, 4, 4], f32)
        nc.vector.memset(c32, 32.0)
        cnh = singles.tile([128, 4, 4], f32)
        nc.vector.memset(cnh, -0.5)

        rho16 = singles.tile([128, 16], f32)
        nc.gpsimd.iota(rho16, [[1, 16]], base=1, channel_multiplier=0,
                       allow_small_or_imprecise_dtypes=True)
        invrho = singles.tile([128, 16], f32)
        nc.vector.reciprocal(invrho, rho16)
        rho16p = singles.tile([128, 2, 16], f32)
        invrhop = singles.tile([128, 2, 16], f32)
        for jj in range(2):
            nc.vector.tensor_copy(rho16p[:, jj], rho16)
            nc.vector.tensor_copy(invrhop[:, jj], invrho)

        fe_r = feat.rearrange("(n c p) k -> n p c k", p=128, c=4)
        pr_r = priors.rearrange("(n c p) d -> n p c d", p=128, c=4)
        out_r = out.rearrange("(n c p) d -> n p c d", p=128, c=4)

        # ---------- main loop (chunk-interleaved software pipeline) ----------
        def emit_load(sc):
            f4 = ft_pool.tile([128, 4, 128], f32, tag="f4")
            nc.sync.dma_start(out=f4, in_=fe_r[sc])
            ftp = ps_ft.tile([128, SC_ROWS], f32, tag="ftp")
            for q in range(4):
                nc.tensor.transpose(ftp[:, q * 128:(q + 1) * 128], f4[:, q],
                                    ident)
            featT = ft_pool.tile([128, SC_ROWS], f32r, tag="featT")
            nc.scalar.copy(featT, ftp)
            p4 = p_pool.tile([128, 4, D], f32)
            nc.sync.dma_start(out=p4, in_=pr_r[sc])
            stats = st_pool.tile([128, 4, 4, 6], f32)
            return dict(featT=featT, p4=p4, stats=stats, xss=[], bn_insts=[])

        def emit_gemm(st, s):
            xp = ps_x.tile([128, SC_ROWS], f32)
            nc.tensor.matmul(xp, WT[:, s * 128:(s + 1) * 128], st["featT"])
            st.setdefault("xps", []).append(xp)

        def emit_copy_bn(st, s):
            xs = xs_pool.tile([128, SC_ROWS], f32)
            nc.scalar.copy(xs, st["xps"][s])
            for c in range(4):
                bi = nc.vector.bn_stats(
                    out=st["stats"][:, s, c],
                    in_=xs[:, c * VBS:(c + 1) * VBS])
                st["bn_insts"].append(bi)
            st["xss"].append(xs)

        def emit_stats(st):
            stats = st["stats"]
            bn_insts = st["bn_insts"]
            me = stats[:, :, :, 1]
            mo = stats[:, :, :, 4]
            M2e = stats[:, :, :, 2]
            M2o = stats[:, :, :, 5]
            dm = sm_pool.tile([128, 4, 4], f32, tag="dm")
            m2 = sm_pool.tile([128, 4, 4], f32, tag="m2")
            sm = sm_pool.tile([128, 4, 4], f32, tag="sm")
            sd = sm_pool.tile([128, 4, 4], f32, tag="sd")
            isd = sm_pool.tile([128, 4, 4], f32, tag="isd")
            sscale = sm_pool.tile([128, 4, 4], f32, tag="sscale")
            tshift = sm_pool.tile([128, 4, 4], f32, tag="tshift")
            i1 = nc.gpsimd.tensor_tensor(dm, me, mo, OP.subtract)
            i2 = nc.gpsimd.tensor_tensor(m2, M2e, M2o, OP.add)
            i3 = nc.gpsimd.tensor_tensor(sm, me, mo, OP.add)
            for bi in bn_insts:
                add_dep_helper(i1.ins, bi.ins, sync=True, reason="stats raw")
                add_dep_helper(i2.ins, bi.ins, sync=True, reason="stats raw")
                add_dep_helper(i3.ins, bi.ins, sync=True, reason="stats raw")
            nc.gpsimd.tensor_tensor(dm, dm, dm, OP.mult)
            nc.gpsimd.tensor_tensor(dm, dm, c32, OP.mult)
            nc.gpsimd.tensor_tensor(m2, dm, m2, OP.add)
            nc.scalar.activation(sd, m2, AF.Sqrt, bias=eps_t, scale=1.0 / VBS)
            nc.vector.reciprocal(isd, sd)
            nc.gpsimd.tensor_tensor(sscale, isd, gamma44, OP.mult)
            nc.gpsimd.tensor_tensor(sm, sm, sscale, OP.mult)
            nc.gpsimd.tensor_tensor(sm, sm, cnh, OP.mult)
            nc.gpsimd.tensor_tensor(tshift, beta44, sm, OP.add)
            st["sscale"] = sscale
            st["tshift"] = tshift

        def emit_chunk_pre(st, c):
            sscale = st["sscale"]
            tshift = st["tshift"]
            xnc = xn_pool.tile([128, 4, VBS], f32, tag="xnc")
            for s in range(4):
                nc.scalar.activation(
                    out=xnc[:, s],
                    in_=st["xss"][s][:, c * VBS:(c + 1) * VBS],
                    func=AF.Identity, bias=tshift[:, s, c:c + 1],
                    scale=sscale[:, s, c:c + 1])
            ztp = ps_zt.tile([128, D], f32)
            for s in range(4):
                nc.tensor.transpose(ztp[:, s * 128:(s + 1) * 128],
                                    xnc[:, s], ident)
            z = z_pool.tile([128, D], f32, tag="z")
            nc.vector.tensor_tensor(z, ztp, st["p4"][:, c], OP.mult)
            st["z"][c] = z

            j = c % 2
            if j == 0:
                st["t16p"] = sm_pool.tile([128, 2, 16], f32, name="t16p", tag="t16p")
                st["csp"] = sm_pool.tile([128, 2, 16], f32, name="csp", tag="csp")
            t16 = st["t16p"]
            cs = st["csp"]
            l1 = sm_pool.tile([128, 32], f32, tag="l1")
            for s in range(4):
                nc.vector.max(out=l1[:, s * 8:(s + 1) * 8],
                              in_=z[:, s * 128:(s + 1) * 128])
            nc.vector.max(out=t16[:, j, 0:8], in_=l1)
            sup = sm_pool.tile([128, 32], f32, tag="sup")
            nc.vector.tensor_scalar(sup, l1, t16[:, j, 7:8], NEG,
                                    OP.is_ge, OP.mult)
            nc.vector.tensor_tensor(sup, l1, sup, OP.add)
            nc.vector.max(out=t16[:, j, 8:16], in_=sup)
            nc.vector.tensor_tensor_scan(cs[:, j], t16[:, j], t16[:, j],
                                         -1.0, OP.add, OP.bypass)

        def emit_pair_tail(st, jp):
            t16 = st["t16p"]
            cs = st["csp"]
            rz = sm_pool.tile([128, 2, 16], f32, tag="rz")
            nc.vector.tensor_tensor(rz, t16, rho16p, OP.mult)
            cond = sm_pool.tile([128, 2, 17], f32, tag="cond")
            nc.gpsimd.memset(cond[:, :, 16:17], 0.0)
            nc.vector.tensor_tensor(cond[:, :, 0:16], rz, cs, OP.is_gt)
            dcn = sm_pool.tile([128, 2, 16], f32, tag="dcn")
            nc.vector.tensor_tensor(dcn, cond[:, :, 1:17], cond[:, :, 0:16],
                                    OP.subtract)
            tauj = sm_pool.tile([128, 2, 16], f32, tag="tauj")
            nc.vector.tensor_tensor(tauj, cs, invrhop, OP.mult)
            scr = sm_pool.tile([128, 2, 16], f32, tag="scr")
            negtau = sm_pool.tile([128, 2], f32, tag="negtau")
            nc.vector.tensor_tensor(scr, tauj, dcn, OP.mult)
            nc.vector.tensor_reduce(out=negtau, in_=scr,
                                    axis=mybir.AxisListType.X, op=OP.add)
            for j in range(2):
                c = 2 * jp + j
                nc.scalar.activation(st["o4"][:, c], st["z"][c], AF.Relu,
                                     bias=negtau[:, j:j + 1], scale=1.0)

        def emit_phase2(pend, sci):
            for i in range(4):
                emit_chunk_pre(pend, i)
                if i % 2 == 1:
                    emit_pair_tail(pend, i // 2)
            nc.sync.dma_start(out=out_r[sci], in_=pend["o4"])

        pend = None
        for sc in range(N_SC):
            st = emit_load(sc)
            if pend is not None:
                pend["o4"] = o_pool.tile([128, 4, D], f32, name="o4", tag="o4")
                pend["z"] = {}
            for i in range(4):
                if pend is not None:
                    emit_chunk_pre(pend, i)
                    if i % 2 == 1:
                        emit_pair_tail(pend, i // 2)
                emit_gemm(st, i)
                emit_copy_bn(st, i)
            if pend is not None:
                nc.sync.dma_start(out=out_r[sc - 1], in_=pend["o4"])
            emit_stats(st)
            pend = st
        pend["o4"] = o_pool.tile([128, 4, D], f32, name="o4", tag="o4")
        pend["z"] = {}
        emit_phase2(pend, N_SC - 1)

    nc.compile()
    return nc


_NC_CACHE = None


def kernel(**inputs) -> np.ndarray:
    global _NC_CACHE
    if _NC_CACHE is None:
        _NC_CACHE = build_nc()
    nc = _NC_CACHE

    priors = np.ascontiguousarray(inputs["priors"], dtype=np.float32)
    feat = np.ascontiguousarray(inputs["processed_feat"], dtype=np.float32)
    W = np.ascontiguousarray(inputs["W"], dtype=np.float32)
    gamma = np.ascontiguousarray(inputs["gamma"], dtype=np.float32)
    beta = np.ascontiguousarray(inputs["beta"], dtype=np.float32)

    in_maps = []
    for i in range(N_CORES):
        sl = slice(i * ROWS, (i + 1) * ROWS)
        in_maps.append({
            "priors": priors[sl],
            "processed_feat": feat[sl],
            "W": W,
            "gamma": gamma,
            "beta": beta,
        })
    res = run_bass_kernel_spmd(nc, in_maps, core_ids=list(range(N_CORES)))
    return np.concatenate([r["out"] for r in res.results], axis=0)


if __name__ == "__main__":
    rng = np.random.default_rng(0)
    inputs = {
        "priors": rng.random((B, D), dtype=np.float32),
        "processed_feat": rng.standard_normal((B, IN), dtype=np.float32),
        "W": (rng.standard_normal((D, IN), dtype=np.float32) * 0.1),
        "gamma": np.ones(D, dtype=np.float32),
        "beta": np.zeros(D, dtype=np.float32),
    }
    out = kernel(**inputs)
    print("out", out.shape, out.dtype, float(out.sum()))

